# revision 5
# baseline (speedup 1.0000x reference)
"""BranchLayer kernel for 8 Trainium2 NeuronCores.

Math: out[b, c] = sum_k x[b, idx[k, c]] * w[k, c], with last-write-wins on
duplicate (idx[k,c], c) pairs — i.e. out = x @ dense where
dense[i, c] = w[k_last, c] for the last k with idx[k, c] == i.

Strategy (sharding_hint): shard the COLS=16384 column dim of dense across the
8 cores (2048 columns each); x is replicated. The host scatters w into dense
(cheap index bookkeeping) and quantizes it to fp8-e3m4 (x stays fp16), so the
dominant HBM stream halves vs fp16 and drops below the TensorE floor of
65536 cycles (~27us @2.4GHz) — the kernel is PE-bound, so the x load is
split into pieces interleaved with the first dense chunks to start the
matmul pipeline (and the PE clock ramp) as early as possible. Output ships
fp16 (values pre-scaled by W_SCALE); the host upcasts and descales.
"""

import numpy as np
import ml_dtypes

import concourse.bass as bass
import concourse.bacc as bacc
import concourse.mybir as mybir
import concourse.tile as tile
from concourse import bass_utils

F16 = np.float16
E3M4 = ml_dtypes.float8_e3m4

# Problem shape (hardcoded per task contract).
N_IN = 4096
N_NPB = 64
N_B = 64
N_NEXT_H = 256
COLS = N_B * N_NEXT_H  # 16384
BATCH = 128
N_CORES = 8

COLS_PER_CORE = COLS // N_CORES  # 2048
N_BLOCK = 512                    # output columns per PSUM block (one bank)
NUM_BLOCKS = COLS_PER_CORE // N_BLOCK  # 4
N_ITILES = N_IN // 128           # 32 contraction tiles

W_SCALE = 64.0                   # dense pre-scale so e3m4 sees ~[-4, 4]
E3M4_MAX = 15.5
X_PIECES = 8                     # x DMA split (512 cols each)

_CACHE = {}


def _build_program(repeats=1, dbufs=4, chunks=8, out_eng="scalar"):
    """One SPMD Bass program; all 8 cores run it on different dense shards.

    repeats>1 loops the whole pipeline inside one NEFF — used only for
    repeat-delta HW timing in test.py (tunnel overhead cancels).
    dbufs: dense-tile pool slots (4 = every block's DMA in flight at start).
    chunks: dense DMA chunks per block (finer ⇒ earlier first matmul and a
    shorter post-DMA tail on the last block).
    """
    if repeats > 1 and out_eng == "scalar":
        # In repeat-timing programs a rep's out-DMAs on the scalar HWDGE
        # queue would stall the NEXT rep's load chunks behind a compute
        # wait; route them via SWDGE there. (Irrelevant for the shipped
        # repeats=1 program — nothing follows its outs.)
        out_eng = "gpsimd"
    key = ("nc", repeats, dbufs, chunks, out_eng)
    if key in _CACHE:
        return _CACHE[key]

    nc = bacc.Bacc(
        "TRN2",
        target_bir_lowering=False,
        debug=False,
        enable_asserts=False,
        num_devices=N_CORES,
    )
    # xT[il, t*128 + b] = x[b, t*128 + il]  (lhsT tiles, fp16)
    xT = nc.dram_tensor("xT", [128, N_IN], mybir.dt.float16, kind="ExternalInput").ap()
    # dns[n, il, t*N_BLOCK + c'] = dense[t*128 + il, n*N_BLOCK + c'] (per-core
    # shard, e3m4, pre-scaled by W_SCALE)
    dns = nc.dram_tensor(
        "dns", [NUM_BLOCKS, 128, N_ITILES * N_BLOCK], mybir.dt.float8e3,
        kind="ExternalInput",
    ).ap()
    out = nc.dram_tensor(
        "out", [BATCH, COLS_PER_CORE], mybir.dt.float16, kind="ExternalOutput"
    ).ap()

    with tile.TileContext(nc) as tc:
        with (
            tc.tile_pool(name="xp", bufs=1) as xp,
            tc.tile_pool(name="dp", bufs=dbufs) as dp,
            tc.tile_pool(name="op", bufs=2) as op,
            tc.tile_pool(name="pp", bufs=2, space="PSUM") as pp,
        ):
            x_sb = xp.tile([128, N_IN], mybir.dt.float16)
            csize = N_ITILES * N_BLOCK // chunks
            xsize = N_IN // X_PIECES
            qs = [nc.sync, nc.scalar]

            for _rep in range(repeats):
                # Block tiles all live at once (dbufs=NUM_BLOCKS) so every
                # load chunk is issued before any compute-dependent
                # instruction lands on the HWDGE queues.
                d_sbs = []
                for _n in range(NUM_BLOCKS):
                    d_sb = dp.tile([128, N_ITILES * N_BLOCK], mybir.dt.float8e3)
                    d_sbs.append(d_sb)
                # The kernel is PE-bound: interleave x pieces with block 0's
                # chunks across the two HWDGE queues so matmul t=0 (needs
                # x piece 0 + d0 chunk 0 only — tile deps are slice-level)
                # starts as soon as the first two transfers land.
                if _rep == 0:
                    for h in range(max(X_PIECES, chunks)):
                        if h < X_PIECES:
                            qs[h % 2].dma_start(
                                out=x_sb[:, h * xsize:(h + 1) * xsize],
                                in_=xT[:, h * xsize:(h + 1) * xsize],
                            )
                        if h < chunks:
                            qs[(h + 1) % 2].dma_start(
                                out=d_sbs[0][:, h * csize:(h + 1) * csize],
                                in_=dns[0, :, h * csize:(h + 1) * csize],
                            )
                    qi = max(X_PIECES, chunks)
                    rest = range(1, NUM_BLOCKS)
                else:
                    qi = 0
                    rest = range(NUM_BLOCKS)
                for n in rest:
                    for h in range(chunks):
                        qs[qi % 2].dma_start(
                            out=d_sbs[n][:, h * csize:(h + 1) * csize],
                            in_=dns[n, :, h * csize:(h + 1) * csize],
                        )
                        qi += 1
                for n in range(NUM_BLOCKS):
                    d_sb = d_sbs[n]
                    ps = pp.tile([BATCH, N_BLOCK], mybir.dt.float32)
                    for t in range(N_ITILES):
                        nc.tensor.matmul(
                            ps[:],
                            x_sb[:, t * 128:(t + 1) * 128],
                            d_sb[:, t * N_BLOCK:(t + 1) * N_BLOCK],
                            start=(t == 0),
                            stop=(t == N_ITILES - 1),
                        )
                    o_sb = op.tile([BATCH, N_BLOCK], mybir.dt.float16)
                    nc.vector.tensor_copy(out=o_sb[:], in_=ps[:])
                    getattr(nc, out_eng).dma_start(
                        out=out[:, n * N_BLOCK:(n + 1) * N_BLOCK], in_=o_sb[:]
                    )

    nc.compile()
    aps = {"xT": xT, "dns": dns, "out": out}
    _CACHE[key] = (nc, aps)
    return nc, aps


def _prepare_inputs(x, w, idx):
    x = np.asarray(x, dtype=np.float32)
    w = np.asarray(w, dtype=np.float32)
    idx = np.asarray(idx)

    # Scatter with last-write-wins (ascending k ⇒ later k overwrites earlier,
    # matching torch's index_put / the reference's keep-mask + scatter-add).
    dense = np.zeros((N_IN, COLS), dtype=np.float32)
    cols = np.arange(COLS)
    for k in range(N_NPB):
        dense[idx[k], cols] = w[k]
    dense = np.clip(dense * W_SCALE, -E3M4_MAX, E3M4_MAX)

    # lhsT layout: xT[il, t, b] = x[b, t*128 + il]
    xT = np.ascontiguousarray(
        x.T.reshape(N_ITILES, 128, BATCH).transpose(1, 0, 2).reshape(128, N_IN)
    ).astype(F16)

    in_maps = []
    for core in range(N_CORES):
        dc = dense[:, core * COLS_PER_CORE:(core + 1) * COLS_PER_CORE]
        # D[n, il, t, c'] = dc[t*128 + il, n*N_BLOCK + c']
        D = np.ascontiguousarray(
            dc.reshape(N_ITILES, 128, NUM_BLOCKS, N_BLOCK)
            .transpose(2, 1, 0, 3)
            .reshape(NUM_BLOCKS, 128, N_ITILES * N_BLOCK)
        ).astype(E3M4)
        in_maps.append({"xT": xT, "dns": D})
    return in_maps


def _run(in_maps, trace=False):
    nc, _ = _build_program()
    res = bass_utils.run_bass_kernel_spmd(
        nc, in_maps, core_ids=list(range(N_CORES)), trace=trace
    )
    _CACHE["last_results"] = res
    return res


def kernel(x, w, idx):
    in_maps = _prepare_inputs(x, w, idx)
    try:
        res = _run(in_maps, trace=False)
    except Exception:
        # A previously wedged device can fail the first attach; one retry
        # on a fresh execution is usually enough (device resets on attach).
        import time
        time.sleep(2.0)
        res = _run(in_maps, trace=False)
    out = np.concatenate(
        [np.asarray(r["out"], dtype=np.float32) for r in res.results], axis=1
    )
    return (out / W_SCALE).reshape(BATCH, N_B, N_NEXT_H).astype(np.float32)
